# revision 1
# baseline (speedup 1.0000x reference)
"""Triplet-margin loss (EuclideanTriple) on 8 Trainium2 NeuronCores.

loss = sum_i relu( ||x_i - y_i + eps||_2 + margin - ||x_i - z_i + eps||_2 )

Data-parallel: N=131072 rows sharded 8 ways (16384 rows/core, no
collectives). Each core reduces its hinge terms to per-partition sums
([128,2]); the host sums the 8 partials into the final scalar.

Per-core layout: rows -> partitions. Chunks of 1024 rows (8 rows per
partition) are loaded as [128, 2048] f32 tiles — each DMA is one contiguous
1 MiB DRAM span with 8 KiB contiguous per-partition writes, quadruple
buffered so the kernel runs at the HBM-read roofline (~48 MiB/core).

Per chunk, compute is split so every engine stays under the DMA time:
  DVE : u = x - y and u' = x - z   (tensor_sub, in place into the y/z tiles)
  ACT : rows 0..3  -> per-row Square(+eps bias) with accum_out = row sum
        rows 4..7  -> one bulk Square(+eps bias)
  DVE : reduce_sum over D for rows 4..7 ([128,4,256] -> [128,4])
The two squared-distance accumulators are separate tiles (one per writing
engine) to avoid cross-engine WAW serialization.
Tail (once per pass): ACT sqrt in place, DVE hinge subtract, ACT
Relu(+margin bias) with accum_out -> per-partition sums, DMA out [128,2].

Measured (For_i-looped, repeat-count slope, incl. ~2-15us loop overhead):
full kernel ~162 us/pass vs DMA-only floor ~159 us -> DMA-bound.
"""

from contextlib import ExitStack

import numpy as np

import concourse.bacc as bacc
import concourse.bass as bass
import concourse.mybir as mybir
import concourse.tile as tile
from concourse import bass_utils

N_TOTAL = 131072
D = 256
N_CORES = 8
SHARD = N_TOTAL // N_CORES  # 16384 rows per core
P = 128                     # SBUF partitions
RPP = SHARD // P            # 128 rows per partition (whole shard)
CHUNK_A = 8                 # rows per partition per chunk (1 MiB DMAs)
N_CHUNKS = RPP // CHUNK_A   # 16 chunks
FD = CHUNK_A * D            # 2048 free-dim elements per chunk tile
MARGIN = 0.5
EPS = 1e-6
F32 = mybir.dt.float32
IO_BUFS = 4
ACT_ROWS = 4  # rows per tensor per chunk whose square+reduce runs on ACT


def build_nc(
    repeat: int = 1,
    mode: str = "full",
    act_rows: int = ACT_ROWS,
    io_bufs: int = IO_BUFS,
    loop: bool = False,
    gp_sub: bool = False,
    chunk_a: int = CHUNK_A,
    act_dma: bool = False,
) -> bass.Bass:
    """mode: 'full' | 'dma' (loads only) | 'compute' (no input loads).
    loop=True wraps the repeats in a For_i hardware loop (for timing runs
    with large repeat counts without unrolled instruction blowup)."""
    
    n_chunks = RPP // chunk_a
    fd = chunk_a * D
    nc = bacc.Bacc("TRN2", target_bir_lowering=False, debug=False)
    x = nc.dram_tensor("x", [SHARD, D], F32, kind="ExternalInput").ap()
    y = nc.dram_tensor("y", [SHARD, D], F32, kind="ExternalInput").ap()
    z = nc.dram_tensor("z", [SHARD, D], F32, kind="ExternalInput").ap()
    # two per-partition partial hinge sums (ACT-rows path, DVE-rows path)
    out = nc.dram_tensor("out", [P, 2], F32, kind="ExternalOutput").ap()

    act = mybir.ActivationFunctionType

    with tile.TileContext(nc) as tc:
        with ExitStack() as ctx:
            io = ctx.enter_context(tc.tile_pool(name="io", bufs=io_bufs))
            acc = ctx.enter_context(tc.tile_pool(name="acc", bufs=1))

            # Per-row squared distances, split into one accumulator per
            # writing engine (a shared tile would WAW-serialize ACT vs DVE):
            #   dsq_act: written by ACT accum_out calls (act_rows per chunk)
            #   dsq_dve: written by DVE tensor_reduce   (dve_rows per chunk)
            # Each is [pos | neg] halves, matching row order between halves.
            dve_rows = chunk_a - act_rows
            na = n_chunks * act_rows   # ACT-path rows per partition
            nd = n_chunks * dve_rows   # DVE-path rows per partition
            dsq_act = acc.tile([P, max(2 * na, 1)], F32, tag="dsq_act")
            dsq_dve = acc.tile([P, max(2 * nd, 1)], F32, tag="dsq_dve")
            # per-partition hinge sums: col 0 = ACT path, col 1 = DVE path
            # (ACT-written only; unwritten column relies on pre-zeroed output)
            hsum = acc.tile([P, 2], F32, tag="hsum")

            # const bias vectors for ACT (bias must be an AP)
            eps_t = acc.tile([P, 1], F32, tag="eps")
            nc.vector.memset(eps_t[:], EPS)
            mar_t = acc.tile([P, 1], F32, tag="mar")
            nc.vector.memset(mar_t[:], MARGIN)

            if mode == "compute":
                # pre-zero both buffer slots of each io tag so compute-only
                # timing reads defined data
                for _ in range(io_bufs):
                    for tag in ("xt", "yt", "zt"):
                        t = io.tile([P, fd], F32, tag=tag)
                        nc.vector.memset(t[:], 0.0)

            def rep_body():
                for c in range(n_chunks):
                    rows = slice(c * P * chunk_a, (c + 1) * P * chunk_a)
                    xt = io.tile([P, fd], F32, tag="xt")
                    yt = io.tile([P, fd], F32, tag="yt")
                    zt = io.tile([P, fd], F32, tag="zt")
                    if mode != "compute":
                        # second HWDGE ring (qActDynamicHW) via the ACT
                        # sequencer when act_dma is set
                        y_eng = nc.scalar if act_dma else nc.sync
                        nc.sync.dma_start(
                            xt[:], x[rows, :].rearrange("(p a) d -> p (a d)", p=P)
                        )
                        y_eng.dma_start(
                            yt[:], y[rows, :].rearrange("(p a) d -> p (a d)", p=P)
                        )
                        nc.sync.dma_start(
                            zt[:], z[rows, :].rearrange("(p a) d -> p (a d)", p=P)
                        )
                    if mode == "dma":
                        continue
                    if mode == "nosq":
                        nc.vector.tensor_sub(yt[:], xt[:], yt[:])
                        nc.vector.tensor_sub(zt[:], xt[:], zt[:])
                        continue
                    if mode == "nored":
                        nc.vector.tensor_sub(yt[:], xt[:], yt[:])
                        nc.vector.tensor_sub(zt[:], xt[:], zt[:])
                        nc.scalar.activation(yt[:], yt[:], act.Square, bias=eps_t[:])
                        nc.scalar.activation(zt[:], zt[:], act.Square, bias=eps_t[:])
                        continue
                    # u = x - y in place into the y/z tiles, then (u + eps)^2
                    # on ACT (the +eps rides ACT's free bias).
                    # Per-row square+reduce is split: the first act_rows rows
                    # of each tile go through per-row ACT calls whose
                    # accum_out directly yields the row's sum; the remaining
                    # rows get one bulk ACT square + a DVE tensor_reduce.
                    nc.vector.tensor_sub(yt[:], xt[:], yt[:])
                    if gp_sub:
                        nc.gpsimd.tensor_sub(zt[:], xt[:], zt[:])
                    else:
                        nc.vector.tensor_sub(zt[:], xt[:], zt[:])
                    for half, t in ((0, yt), (1, zt)):
                        for r in range(act_rows):
                            col = half * na + c * act_rows + r
                            nc.scalar.activation(
                                t[:, r * D : (r + 1) * D],
                                t[:, r * D : (r + 1) * D],
                                act.Square,
                                bias=eps_t[:],
                                accum_out=dsq_act[:, col : col + 1],
                            )
                        if dve_rows:
                            base = half * nd + c * dve_rows
                            nc.scalar.activation(
                                t[:, act_rows * D :],
                                t[:, act_rows * D :],
                                act.Square,
                                bias=eps_t[:],
                            )
                            nc.vector.reduce_sum(
                                dsq_dve[:, base : base + dve_rows],
                                t[:, act_rows * D :].rearrange(
                                    "p (a d) -> p a d", a=dve_rows
                                ),
                                axis=mybir.AxisListType.X,
                            )
                if mode in ("dma", "nosq", "nored"):
                    return

                # tail per accumulator: sqrt (in place), hinge with margin via
                # Relu bias, per-partition sum into its own out column
                for i, (dsq_t, n_cols) in enumerate(
                    ((dsq_act, na), (dsq_dve, nd))
                ):
                    if n_cols == 0:
                        continue
                    nc.scalar.activation(dsq_t[:], dsq_t[:], act.Sqrt)
                    hing = acc.tile([P, n_cols], F32, tag=f"hing{i}")
                    nc.vector.tensor_sub(
                        hing[:], dsq_t[:, :n_cols], dsq_t[:, n_cols:]
                    )
                    relu_t = acc.tile([P, n_cols], F32, tag=f"relu{i}")
                    nc.scalar.activation(
                        relu_t[:],
                        hing[:],
                        act.Relu,
                        bias=mar_t[:],
                        accum_out=hsum[:, i : i + 1],
                    )
                nc.sync.dma_start(out[:], hsum[:])

            if loop and repeat > 1:
                with tc.For_i(0, repeat, 1):
                    rep_body()
            else:
                for _ in range(repeat):
                    rep_body()
    nc.compile()
    return nc


def _run(nc: bass.Bass, x, y, z):
    in_maps = [
        {
            "x": np.ascontiguousarray(x[i * SHARD : (i + 1) * SHARD]),
            "y": np.ascontiguousarray(y[i * SHARD : (i + 1) * SHARD]),
            "z": np.ascontiguousarray(z[i * SHARD : (i + 1) * SHARD]),
        }
        for i in range(N_CORES)
    ]
    return bass_utils.run_bass_kernel_spmd(
        nc, in_maps, core_ids=list(range(N_CORES))
    )


_NC_CACHE = None


def kernel(x: np.ndarray, y: np.ndarray, z: np.ndarray) -> np.ndarray:
    global _NC_CACHE
    x = np.asarray(x, dtype=np.float32)
    y = np.asarray(y, dtype=np.float32)
    z = np.asarray(z, dtype=np.float32)
    if _NC_CACHE is None:
        _NC_CACHE = build_nc(1)
    res = _run(_NC_CACHE, x, y, z)
    total = np.float64(0.0)
    for r in res.results:
        total += r["out"].astype(np.float64).sum()
    return np.float32(total)

